# revision 12
# baseline (speedup 1.0000x reference)
"""EdgeGraphConv on 8 Trainium2 NeuronCores.

Distribution: dst-range sharding. Core c owns destination nodes
[c*N/8, (c+1)*N/8). The host groups edges by (core, dst-tile-of-128,
src-chunk) -- a pure index-space binning -- so each core's segment-sum
is fully local and the final output is a concatenation (no
collectives).

Device algorithm per core:
  phase 0: h = node_feat @ W_node for ALL nodes (replicated work),
           stored to a private HBM table (rows padded to 256B, row
           order swizzled so the store DMA is one contiguous run per
           partition). b_node is folded out algebraically (below).
  phase 2: per super-round (R dst tiles) and src-chunk k: one
           dma_gather (int16 chunk-relative indices) pulls h[src] for
           all that round's chunk-k edges into SBUF; per dst tile a
           one-hot (edge -> dst-local-id, iota+is_equal) matmul
           accumulates in PSUM, in one f32 accumulator:
           S = segsum(h[src]), ef_sum = segsum(edge_feat), deg = count.
  final:   out = (S + ef_sum*W_edge + deg*(b_node+b_edge)) / max(deg,1)
           == mean(h[src]+he) with biases restored; exactly 0 for
           isolated nodes.

The schedule (TILES x NCHUNK x B4 blocks) is data-independent given B4,
so one NEFF serves all 8 cores; per-core differences are pure data.
"""

import sys

for _p in ("/opt/trn_rl_repo", "/opt/pypackages"):
    if _p not in sys.path:
        sys.path.append(_p)

from contextlib import ExitStack

import ml_dtypes
import numpy as np

import concourse.bass as bass
import concourse.mybir as mybir
import concourse.tile as tile
from concourse import bacc, library_config
from concourse.bass_utils import run_bass_kernel_spmd

BF16 = ml_dtypes.bfloat16
N_CORES = 8
P = 128
FE = 128           # padded h-table row elements (256 B)
NCHUNK = 4         # src chunks (chunk row count must fit int16)


def build_bass(B4, K_in, F, TILES, TBL_T, R, PH0_TILES, debug_mode=None):
    """Build the single-NEFF 8-core SPMD bass program.

    B4: 128-edge blocks per (dst-tile, src-chunk);  R: dst tiles per
    super-round (TILES % R == 0);  TBL_T: h-table tiles (global nodes
    padded to TBL_T*128;  must be divisible by NCHUNK).
    """
    NBLK = TILES * NCHUNK * B4
    PAD_N = TBL_T * P
    CH = PAD_N // NCHUNK
    assert TILES % R == 0 and PAD_N % NCHUNK == 0 and CH <= 32768
    NR = TILES // R
    CALL_IDX = R * B4 * P           # indices per dma_gather call

    nc = bacc.Bacc("TRN2", target_bir_lowering=False, debug=False,
                   num_devices=N_CORES)
    dt = mybir.dt

    nfT_d = nc.dram_tensor("nft", [K_in, PAD_N], dt.bfloat16, kind="ExternalInput")
    Wn_d = nc.dram_tensor("wn", [K_in, F], dt.bfloat16, kind="ExternalInput")
    we_d = nc.dram_tensor("we", [1, F], dt.float32, kind="ExternalInput")
    bn_d = nc.dram_tensor("bn", [1, F], dt.float32, kind="ExternalInput")
    be_d = nc.dram_tensor("be", [1, F], dt.float32, kind="ExternalInput")
    iot_d = nc.dram_tensor("iot", [1, P], dt.bfloat16, kind="ExternalInput")
    idx_d = nc.dram_tensor("idx", [P, NBLK * 8], dt.int16, kind="ExternalInput")
    dstl_d = nc.dram_tensor("dstl", [P, TILES, NCHUNK, B4], dt.bfloat16,
                            kind="ExternalInput")
    efo_d = nc.dram_tensor("efo", [P, TILES, NCHUNK, B4, 2], dt.bfloat16,
                           kind="ExternalInput")
    out_d = nc.dram_tensor("out", [TILES * P, F], dt.float32, kind="ExternalOutput")

    # h table row rho = (node % 128) * TBL_T + node // 128  (store is one
    # contiguous DRAM run per partition; gather offsets precomputed in
    # rho space, chunk-relative).
    hkind = {"ph0": "ExternalOutput", "ph2": "ExternalInput"}.get(
        debug_mode, "Internal")
    htbl = nc.dram_tensor("htbl", [PAD_N, FE], dt.bfloat16, kind=hkind)
    htbl_v = htbl.ap().rearrange("(p t) f -> p t f", t=TBL_T)

    mult = mybir.AluOpType.mult
    is_equal = mybir.AluOpType.is_equal

    def emit_phase0(tc):
        with tc.tile_pool(name="ph0", bufs=2) as p0, \
             tc.tile_pool(name="ph0w", bufs=1) as p0w, \
             tc.tile_pool(name="ph0ps", bufs=8, space="PSUM") as p0ps:
            wt = p0w.tile([K_in, F], dt.bfloat16)
            nc.sync.dma_start(out=wt[:], in_=Wn_d.ap())
            for t0 in range(0, TBL_T, PH0_TILES):
                nt = min(PH0_TILES, TBL_T - t0)
                nf_t = p0.tile([K_in, PH0_TILES * P], dt.bfloat16, tag="nf")
                nc.sync.dma_start(out=nf_t[:, :nt * P],
                                  in_=nfT_d.ap()[:, t0 * P:(t0 + nt) * P])
                hst = p0.tile([P, PH0_TILES, FE], dt.bfloat16, tag="hst")
                nc.vector.memset(hst[:, :, F:], 0.0)
                for j0 in range(0, nt, 4):
                    nb = min(4, nt - j0)
                    ps = p0ps.tile([P, 4, F], dt.float32, tag="ps")
                    for j in range(nb):
                        nc.tensor.matmul(
                            ps[:, j, :],
                            lhsT=nf_t[:, (j0 + j) * P:(j0 + j + 1) * P],
                            rhs=wt[:],
                            start=True, stop=True)
                    nc.scalar.copy(out=hst[:, j0:j0 + nb, 0:F],
                                   in_=ps[:, :nb, :])
                nc.sync.dma_start(out=htbl_v[:, t0:t0 + nt, :],
                                  in_=hst[:, :nt, :])

    def emit_phase2(tc, ctx):
        meta = ctx.enter_context(tc.tile_pool(name="meta", bufs=1))
        idx_sb = meta.tile([P, NBLK * 8], dt.int16)
        nc.sync.dma_start(out=idx_sb[:], in_=idx_d.ap())
        dstl_sb = meta.tile([P, TILES, NCHUNK, B4, 1], dt.bfloat16)
        nc.sync.dma_start(out=dstl_sb[:, :, :, :, 0], in_=dstl_d.ap())
        efo_sb = meta.tile([P, TILES, NCHUNK, B4, 2], dt.bfloat16)
        nc.sync.dma_start(out=efo_sb[:], in_=efo_d.ap())

        iota_t = meta.tile([P, 1, 1, P], dt.bfloat16)
        nc.sync.dma_start(out=iota_t[:, 0, :, :],
                          in_=iot_d.ap()[0:1, :].partition_broadcast(P))
        web = meta.tile([P, 1, F], dt.float32)
        nc.sync.dma_start(out=web[:],
                          in_=we_d.ap()[0:1, :].partition_broadcast(P))
        bnb = meta.tile([P, 1, F], dt.float32)
        nc.sync.dma_start(out=bnb[:],
                          in_=bn_d.ap()[0:1, :].partition_broadcast(P))
        beb = meta.tile([P, 1, F], dt.float32)
        nc.sync.dma_start(out=beb[:],
                          in_=be_d.ap()[0:1, :].partition_broadcast(P))
        bb = meta.tile([P, 1, F], dt.float32)
        nc.vector.tensor_add(out=bb[:], in0=bnb[:], in1=beb[:])

        acc = meta.tile([P, TILES, F + 2], dt.float32)

        nc.gpsimd.load_library(library_config.mlp)

        with tc.tile_pool(name="p2", bufs=2) as p2, \
             tc.tile_pool(name="p2oh", bufs=4) as p2oh, \
             tc.tile_pool(name="p2ps", bufs=4, space="PSUM") as p2ps:
            for r in range(NR):
                t0 = r * R
                stages = []
                for k in range(NCHUNK):
                    st = p2.tile([P, R * B4, FE], dt.bfloat16, tag=f"st{k}")
                    col0 = (r * NCHUNK + k) * (CALL_IDX // 16)
                    # SWDGE ring holds ~1024 descriptors per shot; split.
                    SUB = 1024
                    for s0 in range(0, CALL_IDX, SUB):
                        ns = min(SUB, CALL_IDX - s0)
                        nc.gpsimd.dma_gather(
                            out_ap=st[:, s0 // P:(s0 + ns) // P, :],
                            in_ap=htbl.ap()[k * CH:(k + 1) * CH, :],
                            idxs_ap=idx_sb[:, col0 + s0 // 16:
                                           col0 + (s0 + ns) // 16],
                            num_idxs=ns, num_idxs_reg=ns,
                            elem_size=FE)
                    st_v = st[:].rearrange("p (t b) f -> p t b f", b=B4)
                    nc.vector.tensor_copy(
                        out=st_v[:, :, :, F:F + 2],
                        in_=efo_sb[:, t0:t0 + R, k, :, :])
                    stages.append(st)
                for tt in range(R):
                    t = t0 + tt
                    oh = p2oh.tile([P, NCHUNK, B4, P], dt.bfloat16, tag="oh")
                    nc.vector.tensor_tensor(
                        out=oh[:],
                        in0=dstl_sb[:, t, :, :, :].to_broadcast(
                            [P, NCHUNK, B4, P]),
                        in1=iota_t[:].to_broadcast(
                            [P, NCHUNK, B4, P]),
                        op=is_equal)
                    ps2 = p2ps.tile([P, F + 2], dt.float32, tag="ps2")
                    for k in range(NCHUNK):
                        st = stages[k]
                        for b in range(B4):
                            c = tt * B4 + b
                            nc.tensor.matmul(
                                ps2[:],
                                lhsT=oh[:, k, b, :],
                                rhs=st[:, c, 0:F + 2],
                                start=(k == 0 and b == 0),
                                stop=(k == NCHUNK - 1 and b == B4 - 1))
                    nc.scalar.copy(out=acc[:, t, :], in_=ps2[:])

        with tc.tile_pool(name="fin", bufs=1) as fin:
            S = acc[:, :, 0:F]
            ef = acc[:, :, F:F + 1]
            dg = acc[:, :, F + 1:F + 2]
            md = fin.tile([P, TILES, 1], dt.float32)
            nc.vector.tensor_scalar_max(md[:], dg, 1.0)
            rcp = fin.tile([P, TILES, 1], dt.float32)
            nc.vector.reciprocal(out=rcp[:], in_=md[:])
            t1 = fin.tile([P, TILES, F], dt.float32)
            nc.vector.tensor_tensor(out=t1[:],
                                    in0=ef.to_broadcast([P, TILES, F]),
                                    in1=web[:].to_broadcast([P, TILES, F]),
                                    op=mult)
            nc.vector.tensor_add(out=t1[:], in0=t1[:], in1=S)
            t2 = fin.tile([P, TILES, F], dt.float32)
            nc.vector.tensor_tensor(out=t2[:],
                                    in0=dg.to_broadcast([P, TILES, F]),
                                    in1=bb[:].to_broadcast([P, TILES, F]),
                                    op=mult)
            nc.vector.tensor_add(out=t1[:], in0=t1[:], in1=t2[:])
            nc.vector.tensor_tensor(out=t1[:], in0=t1[:],
                                    in1=rcp[:].to_broadcast([P, TILES, F]),
                                    op=mult)
            nc.sync.dma_start(
                out=out_d.ap().rearrange("(p t) f -> p t f", t=TILES),
                in_=t1[:])

    with tile.TileContext(nc) as tc, ExitStack() as ctx:
        if debug_mode != "ph2":
            emit_phase0(tc)
        if debug_mode != "ph0":
            emit_phase2(tc, ctx)
    nc.compile()
    return nc


def _schedule(src, dst, edge_feat, n_nodes, B_override=None):
    """Host-side index-space binning by (core, dst-tile, src-chunk)."""
    E = src.shape[0]
    RN = n_nodes // N_CORES
    TILES = (RN + P - 1) // P
    TBL_T = -(-(n_nodes) // P)
    TBL_T = -(-TBL_T // NCHUNK) * NCHUNK        # divisible by NCHUNK
    PAD_N = TBL_T * P
    CH = PAD_N // NCHUNK

    rho = (src % P) * TBL_T + src // P          # table row of each src
    k = rho // CH
    core = dst // RN
    L = dst - core * RN
    t = L // P
    u = (L % P).astype(np.float32)
    bins = (core * TILES + t) * NCHUNK + k
    nbins = N_CORES * TILES * NCHUNK
    cnt = np.bincount(bins, minlength=nbins)
    B4 = max(1, int(np.max((cnt + P - 1) // P)))
    if B_override is not None:
        B4 = max(B4, B_override)

    order = np.argsort(bins, kind="stable")
    bin_start = np.zeros(nbins, dtype=np.int64)
    np.cumsum(cnt[:-1], out=bin_start[1:])
    rank = np.arange(E, dtype=np.int64) - bin_start[bins[order]]
    dest = bins[order] * (B4 * P) + rank

    SLOTS = nbins * B4 * P
    idxv = np.zeros(SLOTS, dtype=np.int16)         # pad: chunk row 0
    dstl = np.full(SLOTS, -1.0, dtype=np.float32)  # pad: no iota match
    efv = np.zeros(SLOTS, dtype=np.float32)
    one = np.zeros(SLOTS, dtype=np.float32)

    idxv[dest] = (rho - k * CH)[order].astype(np.int16)
    dstl[dest] = u[order]
    efv[dest] = edge_feat[order, 0]
    one[dest] = 1.0

    NBLK = TILES * NCHUNK * B4
    per_core = []
    for c in range(N_CORES):
        sl = slice(c * NBLK * P, (c + 1) * NBLK * P)
        iv = idxv[sl].reshape(TILES, NCHUNK, B4 * P)
        dl = dstl[sl].reshape(TILES, NCHUNK, B4, P).transpose(3, 0, 1, 2)
        eo = np.stack([efv[sl], one[sl]], axis=-1)
        eo = eo.reshape(TILES, NCHUNK, B4, P, 2).transpose(3, 0, 1, 2, 4)
        per_core.append((iv, dl.astype(BF16).copy(), eo.astype(BF16).copy()))
    return per_core, B4, TILES, TBL_T, RN


def _pack_idx(iv, TILES, B4, R):
    """[TILES, NCHUNK, B4*P] chunk-relative rows -> wrapped [P, NBLK*8]."""
    NR = TILES // R
    segs = []
    for r in range(NR):
        for k in range(NCHUNK):
            seq = iv[r * R:(r + 1) * R, k, :].reshape(-1)     # R*B4*128
            segs.append(np.tile(seq.reshape(-1, 16).T, (8, 1)))
    return np.concatenate(segs, axis=1).astype(np.int16)


def _run(node_feat, edge_feat, W_node, b_node, W_edge, b_edge, src, dst,
         r_pref=7, ph0_tiles=98, trace=False, debug_mode=None,
         htbl_in=None):
    n_nodes, K_in = node_feat.shape
    F = W_node.shape[1]
    src = np.asarray(src, dtype=np.int64)
    dst = np.asarray(dst, dtype=np.int64)

    per_core, B4, TILES, TBL_T, RN = _schedule(src, dst, edge_feat, n_nodes)
    R = 1
    for d in range(1, TILES + 1):
        if TILES % d == 0 and d <= r_pref:
            R = d
    PAD_N = TBL_T * P

    nc = build_bass(B4, K_in, F, TILES, TBL_T, R, min(ph0_tiles, TBL_T),
                    debug_mode=debug_mode)

    nfT = np.zeros((K_in, PAD_N), dtype=BF16)
    nfT[:, :n_nodes] = node_feat.T.astype(BF16)
    base = {
        "nft": nfT,
        "wn": W_node.astype(BF16),
        "we": W_edge.astype(np.float32).reshape(1, F),
        "bn": b_node.astype(np.float32).reshape(1, F),
        "be": b_edge.astype(np.float32).reshape(1, F),
        "iot": np.arange(P, dtype=np.float32).reshape(1, P).astype(BF16),
    }
    in_maps = []
    for c in range(N_CORES):
        iv, dl, eo = per_core[c]
        m = dict(base)
        m["idx"] = _pack_idx(iv, TILES, B4, R)
        m["dstl"] = dl
        m["efo"] = eo
        if debug_mode == "ph2":
            m["htbl"] = htbl_in
        in_maps.append(m)

    res = run_bass_kernel_spmd(nc, in_maps, core_ids=list(range(N_CORES)),
                               trace=trace)
    if debug_mode == "ph0":
        return None, res

    # unswizzle: core output row p*TILES + t  ->  local node t*128 + p
    loc = np.arange(RN, dtype=np.int64)
    rows = (loc % P) * TILES + loc // P
    out = np.empty((n_nodes, F), dtype=np.float32)
    for c in range(N_CORES):
        out[c * RN:(c + 1) * RN] = res.results[c]["out"][rows]
    return out, res


def kernel(node_feat, edge_feat, W_node, b_node, W_edge, b_edge, src, dst):
    out, _ = _run(node_feat, edge_feat, W_node, b_node, W_edge, b_edge,
                  src, dst)
    return out
